# revision 21
# baseline (speedup 1.0000x reference)
# Trainium2 Bass kernel for nn_ModelPositional (gnn_message_passing).
#
# Reference computation (B=4, S=512, K=16, V=50265, D=768, DC=784):
#   nodes = pos==0; token = pos>=2
#   emb = emb_table[code]                                  [B,S,768]
#   ntm = nodes[:,None]&token[None,:]&attn  (row-normalized)
#   emb = where(nodes, ntm@emb, emb)
#   pe  = diag(M^t) t=1..16, M = D^-1 A, A = attn|I        [B,S,16]
#   x   = [emb | pe]                                       [B,S,784]
#   out = x @ w_lin + b_lin                                [B,S,50265]
#
# Sharding (8 cores): core c = (batch b=c//2, row-half h=c%2).
#  Stage 1 (per core): masked avg + gather for its 256 rows -> emb-side
#    xT chunks (bf16), AllGather#1 launched immediately (overlaps the
#    RWPE chain); then the 16-step RWPE chain -> pe chunk, AllGather#2
#    (small).  Host permutes token order per core so its rows are always
#    local 0..255 (keeps the SPMD program identical across cores).
#  Stage 2: logits[:, c*6284:(c+1)*6284] = xT.T @ w_pad_c (vocab-shard).
#    w (bf16, host pre-transposed to partition-major) is prefetched into
#    SBUF during stage 1.  Logits are written bf16, host upconverts.
#
# Perf notes:
#  - All DMA layouts are partition-major contiguous (128 descriptors per
#    DMA) so the issuing engine queue isn't swamped pushing descriptors.
#  - DMAs are spread across engine queues (sync: inputs+w+logits,
#    vector: collective inputs, gpsimd: gathers+collective reloads) since
#    a DMA whose wait-semaphore is pending blocks its whole queue.
#  - Tile dependencies are tile-granular, so every multi-writer buffer
#    that feeds the PE (q chain state, m blocks, w chunks, gathered xT)
#    is split into per-block tiles to avoid false cross-block stalls.

import numpy as np

B, S, KPE, V, D = 4, 512, 16, 50265, 768
NCORES = 8
VPAD = 50272          # 8 * 6284
VC = VPAD // NCORES   # 6284 vocab columns per core
DPAD = 896            # 7 * 128 (784 true dims + bias row at 784 + zero pad)
KCH = DPAD // 128     # 7 contraction chunks
KE = 6                # emb-side chunks (dims 0..767); chunk 6 = pe+bias
P = 128
NTOK = B * S          # 2048
MT = NTOK // P        # 16 token tiles

_CACHE = {}


def _build_nc(vc=VC):
    import concourse.bacc as bacc
    import concourse.bass as bass
    import concourse.mybir as mybir
    import concourse.tile as tile
    from concourse.bass import IndirectOffsetOnAxis
    from concourse.masks import make_identity

    f32 = mybir.dt.float32
    bf16 = mybir.dt.bfloat16
    i32 = mybir.dt.int32
    Alu = mybir.AluOpType
    AX = mybir.AxisListType

    nc = bacc.Bacc(
        "TRN2",
        target_bir_lowering=False,
        debug=False,
        num_devices=NCORES,
    )

    attn_in = nc.dram_tensor("attn", [S, S], f32, kind="ExternalInput").ap()
    attnT_in = nc.dram_tensor("attnT", [S, 256], f32, kind="ExternalInput").ap()
    codes_in = nc.dram_tensor("codes", [P, 4], i32, kind="ExternalInput").ap()
    token_in = nc.dram_tensor("token_ps", [P, 4], f32, kind="ExternalInput").ap()
    nodes_in = nc.dram_tensor("nodes_ps", [P, 2], f32, kind="ExternalInput").ap()
    emb_in = nc.dram_tensor("emb_table", [V, D], f32, kind="ExternalInput").ap()
    # host pre-transposed to [p, k-chunk, v] so each k-chunk DMA is one
    # contiguous 2*vc-byte run per partition
    w_in = nc.dram_tensor("w_pad", [P, KCH * vc], bf16, kind="ExternalInput").ap()
    logit_out = nc.dram_tensor("logit", [NTOK, vc], bf16, kind="ExternalOutput").ap()

    n_full, n_rem = divmod(vc, 512)
    ntiles = [512] * n_full + ([n_rem] if n_rem else [])

    with tile.TileContext(nc) as tc:
        with (
            tc.tile_pool(name="persist", bufs=1) as persist,
            tc.tile_pool(name="dram", bufs=1, space="DRAM") as dram,
        ):
            # post-AllGather xT (bf16): emb part split low/high cores so the
            # GEMM can start when the first reload half lands
            xTe_lo = persist.tile([P, 4, KE * 256], bf16, name="xTe_lo")
            xTe_hi = persist.tile([P, 4, KE * 256], bf16, name="xTe_hi")
            xTp_all = persist.tile([P, NCORES, 256], bf16, name="xTp_all")
            # w chunks as separate tiles (per-chunk readiness for the GEMM)
            w_k = [
                persist.tile([P, vc], bf16, name=f"w_k{k}") for k in range(KCH)
            ]

            cce_in = dram.tile([P, KE * 256], bf16, name="cce_in")
            cce_out = dram.tile(
                [NCORES * P, KE * 256], bf16, name="cce_out", addr_space="Shared"
            )
            ccp_in = dram.tile([P, 256], bf16, name="ccp_in")
            ccp_out = dram.tile(
                [NCORES * P, 256], bf16, name="ccp_out", addr_space="Shared"
            )
            ccw_in = dram.tile([P, 16], bf16, name="ccw_in")
            ccw_out = dram.tile(
                [NCORES * P, 16], bf16, name="ccw_out", addr_space="Shared"
            )

            # ---------------- Stage 1 ----------------
            with (
                tc.tile_pool(name="s1", bufs=1) as s1,
                tc.tile_pool(name="s1tmp", bufs=2) as s1t,
                tc.tile_pool(name="psq", bufs=4, space="PSUM") as psq,
                tc.tile_pool(name="psa", bufs=2, space="PSUM") as psa,
                tc.tile_pool(name="pst", bufs=2, space="PSUM") as pst,
            ):
                # --- input DMAs (sync queue) ---
                codes_sb = s1.tile([P, 4], i32, name="codes_sb")
                nc.sync.dma_start(out=codes_sb[:], in_=codes_in)
                token_sb = s1.tile([P, 4], f32, name="token_sb")
                nc.sync.dma_start(out=token_sb[:], in_=token_in)
                nodes_sb = s1.tile([P, 2], f32, name="nodes_sb")
                nc.sync.dma_start(out=nodes_sb[:], in_=nodes_in)
                attn_sb = s1.tile([P, 4, S], f32, name="attn_sb")
                nc.sync.dma_start(
                    out=attn_sb[:], in_=attn_in.rearrange("(j p) s -> p j s", p=P)
                )
                attnT_sb = s1.tile([P, 4, 256], f32, name="attnT_sb")
                nc.sync.dma_start(
                    out=attnT_sb[:], in_=attnT_in.rearrange("(k p) r -> p k r", p=P)
                )

                # warm-up collective: pays the CC first-trigger latency and
                # starts the inter-core barrier as early as possible
                nc.gpsimd.collective_compute(
                    "AllGather",
                    mybir.AluOpType.bypass,
                    replica_groups=[list(range(NCORES))],
                    ins=[ccw_in[:].opt()],
                    outs=[ccw_out[:].opt()],
                )

                # --- embedding gather: per-token-block tiles ---
                emb_plus = [
                    s1.tile([P, D + 2], f32, name=f"emb_plus{k}") for k in range(4)
                ]
                for k in range(4):
                    nc.vector.memset(emb_plus[k][:, D : D + 1], 1.0)
                    nc.vector.memset(emb_plus[k][:, D + 1 : D + 2], 0.0)
                for k in range(4):
                    nc.gpsimd.indirect_dma_start(
                        out=emb_plus[k][:, 0:D],
                        out_offset=None,
                        in_=emb_in[:, :],
                        in_offset=IndirectOffsetOnAxis(
                            ap=codes_sb[:, k : k + 1], axis=0
                        ),
                    )

                ident = s1.tile([P, P], f32, name="ident")
                make_identity(nc, ident[:])

                # --- gpsimd masks: force attn diagonal to 1 (A = attn|I) ---
                for j in range(4):
                    nc.gpsimd.affine_select(
                        out=attn_sb[:, j, :],
                        in_=attn_sb[:, j, :],
                        compare_op=Alu.not_equal,
                        fill=1.0,
                        base=j * P,
                        pattern=[[-1, S]],
                        channel_multiplier=1,
                    )
                # eye blocks for diag extraction (rows 0..255 only)
                eye2 = [s1.tile([P, 256], f32, name=f"eye2_{i}") for i in range(2)]
                for i in range(2):
                    nc.gpsimd.memset(eye2[i][:], 0.0)
                    nc.gpsimd.affine_select(
                        out=eye2[i][:],
                        in_=eye2[i][:],
                        compare_op=Alu.not_equal,
                        fill=1.0,
                        base=i * P,
                        pattern=[[-1, 256]],
                        channel_multiplier=1,
                    )
                # Q0 = I[:, 0:256] as four per-block tiles
                qA = [s1.tile([P, 256], bf16, name=f"qA{j}") for j in range(4)]
                qB = [s1.tile([P, 256], bf16, name=f"qB{j}") for j in range(4)]
                for j in range(4):
                    nc.gpsimd.memset(qA[j][:], 0.0)
                for j in range(2):
                    nc.gpsimd.affine_select(
                        out=qA[j][:],
                        in_=qA[j][:],
                        compare_op=Alu.not_equal,
                        fill=1.0,
                        base=j * P,
                        pattern=[[-1, 256]],
                        channel_multiplier=1,
                    )
                pe_pad = s1.tile([P, 2, P], f32, name="pe_pad")
                nc.gpsimd.memset(pe_pad[:], 0.0)
                nc.gpsimd.memset(pe_pad[:, :, KPE : KPE + 1], 1.0)

                # --- masked-average path (feeds AllGather#1 asap) ---
                ntmT = [s1.tile([P, 256], bf16, name=f"ntmT{k}") for k in range(4)]
                for k in range(4):
                    nc.vector.tensor_scalar_mul(
                        ntmT[k][:], attnT_sb[:, k, :], token_sb[:, k : k + 1]
                    )
                emb_b = [
                    s1.tile([P, D + 2], bf16, name=f"emb_b{k}") for k in range(4)
                ]
                for k in range(4):
                    nc.vector.tensor_copy(out=emb_b[k][:], in_=emb_plus[k][:])

                # avg rows (for our 256 rows) + row-sum via the ones column
                embo = [s1.tile([P, D], f32, name=f"embo{lj}") for lj in range(2)]
                for lj in range(2):
                    pav0 = psa.tile([P, 512], f32, tag="pav")
                    pav1 = psa.tile([P, 512], f32, tag="pav")
                    for k in range(4):
                        nc.tensor.matmul(
                            out=pav0[:],
                            lhsT=ntmT[k][:, lj * P : (lj + 1) * P],
                            rhs=emb_b[k][:, 0:512],
                            start=(k == 0),
                            stop=(k == 3),
                        )
                    for k in range(4):
                        nc.tensor.matmul(
                            out=pav1[:, 0 : D + 2 - 512],
                            lhsT=ntmT[k][:, lj * P : (lj + 1) * P],
                            rhs=emb_b[k][:, 512 : D + 2],
                            start=(k == 0),
                            stop=(k == 3),
                        )
                    rs = s1t.tile([P, 1], f32, tag="rs")
                    nc.vector.tensor_scalar_add(rs[:], pav1[:, D - 512 : D - 511], 1e-10)
                    rinv = s1t.tile([P, 1], f32, tag="rinv")
                    nc.vector.reciprocal(out=rinv[:], in_=rs[:])
                    avg_n = s1t.tile([P, D], f32, tag="avg_n")
                    nc.vector.tensor_scalar_mul(avg_n[:, 0:512], pav0[:], rinv[:])
                    nc.vector.tensor_scalar_mul(
                        avg_n[:, 512:D], pav1[:, 0 : D - 512], rinv[:]
                    )
                    # emb_out = emb + nodes * (avg - emb)
                    d1 = s1t.tile([P, D], f32, tag="d1")
                    nc.vector.tensor_sub(d1[:], avg_n[:], emb_plus[lj][:, 0:D])
                    nc.vector.tensor_scalar_mul(d1[:], d1[:], nodes_sb[:, lj : lj + 1])
                    nc.vector.tensor_add(embo[lj][:], emb_plus[lj][:, 0:D], d1[:])

                # transpose emb side -> xT_emb_loc [128, 6*256] bf16 (flat)
                xTe_loc = s1.tile([P, KE * 256], bf16, name="xTe_loc")
                for lj in range(2):
                    for kk in range(KE):
                        pt = pst.tile([P, P], f32, tag="pt")
                        nc.tensor.transpose(
                            out=pt[:],
                            in_=embo[lj][:, kk * P : (kk + 1) * P],
                            identity=ident[:],
                        )
                        o0 = kk * 256 + lj * P
                        if (lj * KE + kk) % 2 == 0:
                            nc.scalar.copy(out=xTe_loc[:, o0 : o0 + P], in_=pt[:])
                        else:
                            nc.vector.tensor_copy(
                                out=xTe_loc[:, o0 : o0 + P], in_=pt[:]
                            )

                # --- M = A/deg as per-block bf16 tiles (vector) ---
                m_sb = [s1.tile([P, S], bf16, name=f"m_sb{j}") for j in range(4)]
                for j in range(4):
                    deg_j = s1t.tile([P, 1], f32, tag="deg_j")
                    nc.vector.reduce_sum(out=deg_j[:], in_=attn_sb[:, j, :], axis=AX.X)
                    rinv_j = s1t.tile([P, 1], f32, tag="rinv_j")
                    nc.vector.reciprocal(out=rinv_j[:], in_=deg_j[:])
                    nc.vector.tensor_scalar_mul(m_sb[j][:], attn_sb[:, j, :], rinv_j[:])

                # --- AllGather #1 (emb xT) + split reload ---
                nc.scalar.dma_start(out=cce_in[:, :], in_=xTe_loc[:])
                # w prefetch AFTER the latency-critical inputs: issuing from
                # the scalar queue here keeps its 11 MB off the HBM during
                # the first ~25 us (gathers/attn/attnT are latency-critical)
                for k in range(KCH):
                    nc.scalar.dma_start(
                        out=w_k[k][:], in_=w_in[:, k * vc : (k + 1) * vc]
                    )
                nc.gpsimd.collective_compute(
                    "AllGather",
                    mybir.AluOpType.bypass,
                    replica_groups=[list(range(NCORES))],
                    ins=[cce_in[:].opt()],
                    outs=[cce_out[:].opt()],
                )
                cce_re = cce_out.rearrange("(c p) f -> p c f", p=P)
                nc.gpsimd.dma_start(out=xTe_lo[:], in_=cce_re[:, 0:4, :])
                nc.gpsimd.dma_start(out=xTe_hi[:], in_=cce_re[:, 4:8, :])

                # --- RWPE chain: Q_{t+1} = M^T Q_t, diag from rows 0..255 ---
                cur, nxt = qA, qB
                for t in range(KPE):
                    for i in range(4):
                        pq = psq.tile([P, 256], f32, tag="pq")
                        for j in range(4):
                            nc.tensor.matmul(
                                out=pq[:],
                                lhsT=m_sb[j][:, i * P : (i + 1) * P],
                                rhs=cur[j][:],
                                start=(j == 0),
                                stop=(j == 3),
                            )
                        if i < 2:
                            # diag of this 128-row block: E mask = eye slice
                            # (tensor_tensor_reduce crashes on HW; use mul+red)
                            dummy = s1t.tile([P, 256], f32, tag="ttr_dummy")
                            nc.vector.tensor_mul(dummy[:], pq[:], eye2[i][:])
                            nc.vector.reduce_sum(
                                out=pe_pad[:, i, t : t + 1], in_=dummy[:], axis=AX.X
                            )
                        if t < KPE - 1:
                            # all copies on scalar: vector is busy with the
                            # diag extraction (2 muls + 2 reduces per step)
                            nc.scalar.copy(out=nxt[i][:], in_=pq[:])
                    cur, nxt = nxt, cur

                # pe transpose -> xT_pe_loc [128, 256] bf16 (+AllGather#2)
                xTp_loc = s1.tile([P, 256], bf16, name="xTp_loc")
                for lj in range(2):
                    pt2 = pst.tile([P, P], f32, tag="pt")
                    nc.tensor.transpose(
                        out=pt2[:], in_=pe_pad[:, lj, :], identity=ident[:]
                    )
                    nc.vector.tensor_copy(
                        out=xTp_loc[:, lj * P : (lj + 1) * P], in_=pt2[:]
                    )
                nc.scalar.dma_start(out=ccp_in[:, :], in_=xTp_loc[:])
                nc.gpsimd.collective_compute(
                    "AllGather",
                    mybir.AluOpType.bypass,
                    replica_groups=[list(range(NCORES))],
                    ins=[ccp_in[:].opt()],
                    outs=[ccp_out[:].opt()],
                )
                nc.gpsimd.dma_start(
                    out=xTp_all[:],
                    in_=ccp_out.rearrange("(c p) r -> p c r", p=P),
                )

            # ---------------- Stage 2: logits = xT.T @ w ----------------
            def lhsT_for(m, k):
                rc, lj = divmod(m, 2)
                if k < KE:
                    xte = xTe_lo if rc < 4 else xTe_hi
                    o0 = k * 256 + lj * P
                    return xte[:, rc % 4, o0 : o0 + P]
                return xTp_all[:, rc, lj * P : (lj + 1) * P]

            with (
                tc.tile_pool(name="ob", bufs=4) as obp,
                tc.tile_pool(name="ps2", bufs=8, space="PSUM") as ps2,
            ):
                cp_i = 0

                def copyout(po, m, n0, ntn):
                    nonlocal cp_i
                    ob = obp.tile([P, 512], bf16, tag="ob")
                    if cp_i % 2 == 0:
                        nc.scalar.copy(out=ob[:, 0:ntn], in_=po[:, 0:ntn])
                    else:
                        nc.vector.tensor_copy(out=ob[:, 0:ntn], in_=po[:, 0:ntn])
                    cp_i += 1
                    nc.sync.dma_start(
                        out=logit_out[m * P : (m + 1) * P, n0 : n0 + ntn],
                        in_=ob[:, 0:ntn],
                    )

                for n, ntn in enumerate(ntiles):
                    n0 = n * 512
                    if n == 0:
                        # first v-tile: run k0..5 for 8 token tiles before any
                        # k6, so the PE isn't stalled on the (late) pe
                        # AllGather when the GEMM starts.
                        holds = []
                        for m in range(8):
                            po = ps2.tile([P, 512], f32, tag="po")
                            for k in range(KE):
                                nc.tensor.matmul(
                                    out=po[:, 0:ntn],
                                    lhsT=lhsT_for(m, k),
                                    rhs=w_k[k][:, n0 : n0 + ntn],
                                    start=(k == 0),
                                    stop=False,
                                )
                            holds.append(po)
                        for m in range(8):
                            po = holds[m]
                            nc.tensor.matmul(
                                out=po[:, 0:ntn],
                                lhsT=lhsT_for(m, KE),
                                rhs=w_k[KE][:, n0 : n0 + ntn],
                                start=False,
                                stop=True,
                            )
                            copyout(po, m, n0, ntn)
                        rest = range(8, MT)
                    else:
                        rest = range(MT)
                    for m in rest:
                        po = ps2.tile([P, 512], f32, tag="po")
                        for k in range(KCH):
                            nc.tensor.matmul(
                                out=po[:, 0:ntn],
                                lhsT=lhsT_for(m, k),
                                rhs=w_k[k][:, n0 : n0 + ntn],
                                start=(k == 0),
                                stop=(k == KCH - 1),
                            )
                        copyout(po, m, n0, ntn)

    nc.compile()
    return nc


def _host_prep(code_inputs, position_idx, attn_mask, emb_table, w_lin, b_lin, vc=VC):
    import ml_dtypes

    code = np.asarray(code_inputs).astype(np.int32)
    pos = np.asarray(position_idx).astype(np.int32)
    attn = np.asarray(attn_mask).astype(np.float32)
    emb_t = np.ascontiguousarray(np.asarray(emb_table, dtype=np.float32))
    w = np.asarray(w_lin, dtype=np.float32)
    bias = np.asarray(b_lin, dtype=np.float32)

    w_ext = np.zeros((DPAD, NCORES * vc), np.float32)
    ncols = min(NCORES * vc, V)
    w_ext[: D + KPE, :ncols] = w[:, :ncols]
    w_ext[D + KPE, :ncols] = bias[:ncols]
    w_ext = w_ext.astype(ml_dtypes.bfloat16)

    nodes = (pos == 0).astype(np.float32)
    token = (pos >= 2).astype(np.float32)

    in_maps = []
    for c in range(NCORES):
        b, h = divmod(c, 2)
        if h == 0:
            perm = np.arange(S)
        else:
            perm = np.r_[256:512, 0:256]
        a_p = attn[b][perm][:, perm]
        # [896, vc] -> [128 p, 7 k * vc] partition-major for contiguous DMA
        w_c = (
            w_ext[:, c * vc : (c + 1) * vc]
            .reshape(KCH, P, vc)
            .transpose(1, 0, 2)
            .reshape(P, KCH * vc)
        )
        in_maps.append(
            {
                "attn": np.ascontiguousarray(a_p),
                "attnT": np.ascontiguousarray(a_p[:256, :].T),
                "codes": np.ascontiguousarray(code[b][perm].reshape(4, P).T),
                "token_ps": np.ascontiguousarray(token[b][perm].reshape(4, P).T),
                "nodes_ps": np.ascontiguousarray(
                    nodes[b][perm][:256].reshape(2, P).T
                ),
                "emb_table": emb_t,
                "w_pad": np.ascontiguousarray(w_c),
            }
        )
    return in_maps


def run(inputs, trace=False, vc=VC, **run_kwargs):
    from concourse.bass_utils import run_bass_kernel_spmd

    key = ("nc", vc)
    nc = _CACHE.get(key)
    if nc is None:
        nc = _build_nc(vc=vc)
        _CACHE[key] = nc
    in_maps = _host_prep(**inputs, vc=vc)
    res = run_bass_kernel_spmd(
        nc, in_maps, core_ids=list(range(NCORES)), trace=trace, **run_kwargs
    )
    ncols = min(NCORES * vc, V)
    logits = np.concatenate(
        [np.asarray(r["logit"]).astype(np.float32) for r in res.results], axis=1
    )[:, :ncols]
    return logits.reshape(B, S, ncols), res


def kernel(**inputs):
    logits, _ = run(inputs, trace=False)
    return logits


# revision 27
# speedup vs baseline: 1.0157x; 1.0157x over previous
# Trainium2 Bass kernel for nn_ModelPositional (gnn_message_passing).
#
# Reference computation (B=4, S=512, K=16, V=50265, D=768, DC=784):
#   nodes = pos==0; token = pos>=2
#   emb = emb_table[code]                                  [B,S,768]
#   ntm = nodes[:,None]&token[None,:]&attn  (row-normalized)
#   emb = where(nodes, ntm@emb, emb)
#   pe  = diag(M^t) t=1..16, M = D^-1 A, A = attn|I        [B,S,16]
#   x   = [emb | pe]                                       [B,S,784]
#   out = x @ w_lin + b_lin                                [B,S,50265]
#
# Sharding (8 cores): core c = (batch b=c//2, row-half h=c%2).
#  Stage 1 (per core): masked avg + gather for its 256 rows -> emb-side
#    xT chunks (bf16), AllGather#1 launched immediately (overlaps the
#    RWPE chain); then the 16-step RWPE chain -> pe chunk, AllGather#2
#    (small).  Host permutes token order per core so its rows are always
#    local 0..255 (keeps the SPMD program identical across cores).
#  Stage 2: logits[:, c*6284:(c+1)*6284] = xT.T @ w_pad_c (vocab-shard).
#    w (bf16, host pre-transposed to partition-major) is prefetched into
#    SBUF during stage 1.  Logits are written bf16, host upconverts.
#
# Perf notes:
#  - All DMA layouts are partition-major contiguous (128 descriptors per
#    DMA) so the issuing engine queue isn't swamped pushing descriptors.
#  - DMAs are spread across engine queues (sync: inputs+w+logits,
#    vector: collective inputs, gpsimd: gathers+collective reloads) since
#    a DMA whose wait-semaphore is pending blocks its whole queue.
#  - Tile dependencies are tile-granular, so every multi-writer buffer
#    that feeds the PE (q chain state, m blocks, w chunks, gathered xT)
#    is split into per-block tiles to avoid false cross-block stalls.

import numpy as np

B, S, KPE, V, D = 4, 512, 16, 50265, 768
NCORES = 8
VPAD = 50272          # 8 * 6284
VC = VPAD // NCORES   # 6284 vocab columns per core
DPAD = 896            # 7 * 128 (784 true dims + bias row at 784 + zero pad)
KCH = DPAD // 128     # 7 contraction chunks
KE = 6                # emb-side chunks (dims 0..767); chunk 6 = pe+bias
P = 128
NTOK = B * S          # 2048
MT = NTOK // P        # 16 token tiles

_CACHE = {}


def _build_nc(vc=VC):
    import concourse.bacc as bacc
    import concourse.bass as bass
    import concourse.mybir as mybir
    import concourse.tile as tile
    from concourse.bass import IndirectOffsetOnAxis
    from concourse.masks import make_identity

    f32 = mybir.dt.float32
    bf16 = mybir.dt.bfloat16
    i32 = mybir.dt.int32
    Alu = mybir.AluOpType
    AX = mybir.AxisListType

    nc = bacc.Bacc(
        "TRN2",
        target_bir_lowering=False,
        debug=False,
        num_devices=NCORES,
    )

    attn_in = nc.dram_tensor("attn", [S, S], f32, kind="ExternalInput").ap()
    attnT_in = nc.dram_tensor("attnT", [S, 256], f32, kind="ExternalInput").ap()
    codes_in = nc.dram_tensor("codes", [P, 4], i32, kind="ExternalInput").ap()
    token_in = nc.dram_tensor("token_ps", [P, 4], f32, kind="ExternalInput").ap()
    nodes_in = nc.dram_tensor("nodes_ps", [P, 2], f32, kind="ExternalInput").ap()
    emb_in = nc.dram_tensor("emb_table", [V, D], f32, kind="ExternalInput").ap()
    # host pre-transposed to [p, k-chunk, v] so each k-chunk DMA is one
    # contiguous 2*vc-byte run per partition
    w_in = nc.dram_tensor("w_pad", [P, KCH * vc], bf16, kind="ExternalInput").ap()
    logit_out = nc.dram_tensor("logit", [NTOK, vc], bf16, kind="ExternalOutput").ap()

    n_full, n_rem = divmod(vc, 512)
    ntiles = [512] * n_full + ([n_rem] if n_rem else [])

    with tile.TileContext(nc) as tc:
        with (
            tc.tile_pool(name="persist", bufs=1) as persist,
            tc.tile_pool(name="dram", bufs=1, space="DRAM") as dram,
        ):
            # post-AllGather xT (bf16): emb part split low/high cores so the
            # GEMM can start when the first reload half lands
            xTe_lo = persist.tile([P, 4, KE * 256], bf16, name="xTe_lo")
            xTe_hi = persist.tile([P, 4, KE * 256], bf16, name="xTe_hi")
            xTp_all = persist.tile([P, NCORES, 256], bf16, name="xTp_all")
            # w chunks as separate tiles (per-chunk readiness for the GEMM)
            w_k = [
                persist.tile([P, vc], bf16, name=f"w_k{k}") for k in range(KCH)
            ]

            cce_in = dram.tile([P, KE * 256], bf16, name="cce_in")
            cce_out = dram.tile(
                [NCORES * P, KE * 256], bf16, name="cce_out", addr_space="Shared"
            )
            ccp_in = dram.tile([P, 256], bf16, name="ccp_in")
            ccp_out = dram.tile(
                [NCORES * P, 256], bf16, name="ccp_out", addr_space="Shared"
            )


            # ---------------- Stage 1 ----------------
            with (
                tc.tile_pool(name="s1", bufs=1) as s1,
                tc.tile_pool(name="s1tmp", bufs=2) as s1t,
                tc.tile_pool(name="psq", bufs=3, space="PSUM") as psq,
                tc.tile_pool(name="psa", bufs=2, space="PSUM") as psa,
                tc.tile_pool(name="pst", bufs=2, space="PSUM") as pst,
            ):
                # --- input DMAs (sync queue) ---
                codes_sb = s1.tile([P, 4], i32, name="codes_sb")
                nc.sync.dma_start(out=codes_sb[:], in_=codes_in)
                token_sb = s1.tile([P, 4], f32, name="token_sb")
                nc.sync.dma_start(out=token_sb[:], in_=token_in)
                nodes_sb = s1.tile([P, 2], f32, name="nodes_sb")
                nc.sync.dma_start(out=nodes_sb[:], in_=nodes_in)
                attn_sb = s1.tile([P, 4, S], f32, name="attn_sb")
                nc.sync.dma_start(
                    out=attn_sb[:], in_=attn_in.rearrange("(j p) s -> p j s", p=P)
                )
                attnT_sb = s1.tile([P, 4, 256], f32, name="attnT_sb")
                nc.sync.dma_start(
                    out=attnT_sb[:], in_=attnT_in.rearrange("(k p) r -> p k r", p=P)
                )

                # --- embedding gather: per-token-block tiles ---
                emb_plus = [
                    s1.tile([P, D + 2], f32, name=f"emb_plus{k}") for k in range(4)
                ]
                for k in range(4):
                    nc.vector.memset(emb_plus[k][:, D : D + 1], 1.0)
                    nc.vector.memset(emb_plus[k][:, D + 1 : D + 2], 0.0)
                for k in range(4):
                    nc.gpsimd.indirect_dma_start(
                        out=emb_plus[k][:, 0:D],
                        out_offset=None,
                        in_=emb_in[:, :],
                        in_offset=IndirectOffsetOnAxis(
                            ap=codes_sb[:, k : k + 1], axis=0
                        ),
                    )

                # --- w prefetch: 7 contiguous chunk DMAs (sync queue) ---
                for k in range(KCH):
                    nc.sync.dma_start(
                        out=w_k[k][:], in_=w_in[:, k * vc : (k + 1) * vc]
                    )

                ident = s1.tile([P, P], f32, name="ident")
                make_identity(nc, ident[:])

                # --- gpsimd masks: force attn diagonal to 1 (A = attn|I) ---
                for j in range(4):
                    nc.gpsimd.affine_select(
                        out=attn_sb[:, j, :],
                        in_=attn_sb[:, j, :],
                        compare_op=Alu.not_equal,
                        fill=1.0,
                        base=j * P,
                        pattern=[[-1, S]],
                        channel_multiplier=1,
                    )
                # eye blocks for diag extraction (rows 0..255 only)
                eye2 = [s1.tile([P, 256], f32, name=f"eye2_{i}") for i in range(2)]
                for i in range(2):
                    nc.gpsimd.memset(eye2[i][:], 0.0)
                    nc.gpsimd.affine_select(
                        out=eye2[i][:],
                        in_=eye2[i][:],
                        compare_op=Alu.not_equal,
                        fill=1.0,
                        base=i * P,
                        pattern=[[-1, 256]],
                        channel_multiplier=1,
                    )
                # Q0 = I[:, 0:256] as four per-block tiles
                qA = [s1.tile([P, 256], bf16, name=f"qA{j}") for j in range(4)]
                qB = [s1.tile([P, 256], bf16, name=f"qB{j}") for j in range(4)]
                for j in range(4):
                    nc.gpsimd.memset(qA[j][:], 0.0)
                for j in range(2):
                    nc.gpsimd.affine_select(
                        out=qA[j][:],
                        in_=qA[j][:],
                        compare_op=Alu.not_equal,
                        fill=1.0,
                        base=j * P,
                        pattern=[[-1, 256]],
                        channel_multiplier=1,
                    )
                pe_pad = s1.tile([P, 2, P], f32, name="pe_pad")
                nc.gpsimd.memset(pe_pad[:], 0.0)
                nc.gpsimd.memset(pe_pad[:, :, KPE : KPE + 1], 1.0)

                # --- masked-average path (feeds AllGather#1 asap) ---
                ntmT = [s1.tile([P, 256], bf16, name=f"ntmT{k}") for k in range(4)]
                for k in range(4):
                    nc.vector.tensor_scalar_mul(
                        ntmT[k][:], attnT_sb[:, k, :], token_sb[:, k : k + 1]
                    )
                emb_b = [
                    s1.tile([P, D + 2], bf16, name=f"emb_b{k}") for k in range(4)
                ]
                for k in range(4):
                    nc.vector.tensor_copy(out=emb_b[k][:], in_=emb_plus[k][:])

                # avg rows (for our 256 rows) + row-sum via the ones column
                embo = [s1.tile([P, D], f32, name=f"embo{lj}") for lj in range(2)]
                for lj in range(2):
                    pav0 = psa.tile([P, 512], f32, tag="pav")
                    pav1 = psa.tile([P, 512], f32, tag="pav")
                    for k in range(4):
                        nc.tensor.matmul(
                            out=pav0[:],
                            lhsT=ntmT[k][:, lj * P : (lj + 1) * P],
                            rhs=emb_b[k][:, 0:512],
                            start=(k == 0),
                            stop=(k == 3),
                        )
                    for k in range(4):
                        nc.tensor.matmul(
                            out=pav1[:, 0 : D + 2 - 512],
                            lhsT=ntmT[k][:, lj * P : (lj + 1) * P],
                            rhs=emb_b[k][:, 512 : D + 2],
                            start=(k == 0),
                            stop=(k == 3),
                        )
                    rs = s1t.tile([P, 1], f32, tag="rs")
                    nc.vector.tensor_scalar_add(rs[:], pav1[:, D - 512 : D - 511], 1e-10)
                    rinv = s1t.tile([P, 1], f32, tag="rinv")
                    nc.vector.reciprocal(out=rinv[:], in_=rs[:])
                    avg_n = s1t.tile([P, D], f32, tag="avg_n")
                    nc.vector.tensor_scalar_mul(avg_n[:, 0:512], pav0[:], rinv[:])
                    nc.vector.tensor_scalar_mul(
                        avg_n[:, 512:D], pav1[:, 0 : D - 512], rinv[:]
                    )
                    # emb_out = emb + nodes * (avg - emb)
                    d1 = s1t.tile([P, D], f32, tag="d1")
                    nc.vector.tensor_sub(d1[:], avg_n[:], emb_plus[lj][:, 0:D])
                    nc.vector.tensor_scalar_mul(d1[:], d1[:], nodes_sb[:, lj : lj + 1])
                    nc.vector.tensor_add(embo[lj][:], emb_plus[lj][:, 0:D], d1[:])

                # transpose emb side -> xT_emb_loc [128, 6*256] bf16 (flat)
                xTe_loc = s1.tile([P, KE * 256], bf16, name="xTe_loc")
                for lj in range(2):
                    for kk in range(KE):
                        pt = pst.tile([P, P], f32, tag="pt")
                        nc.tensor.transpose(
                            out=pt[:],
                            in_=embo[lj][:, kk * P : (kk + 1) * P],
                            identity=ident[:],
                        )
                        o0 = kk * 256 + lj * P
                        if (lj * KE + kk) % 2 == 0:
                            nc.scalar.copy(out=xTe_loc[:, o0 : o0 + P], in_=pt[:])
                        else:
                            nc.vector.tensor_copy(
                                out=xTe_loc[:, o0 : o0 + P], in_=pt[:]
                            )

                # --- M = A/deg as per-block bf16 tiles (vector) ---
                m_sb = [s1.tile([P, S], bf16, name=f"m_sb{j}") for j in range(4)]
                for j in range(4):
                    deg_j = s1t.tile([P, 1], f32, tag="deg_j")
                    nc.vector.reduce_sum(out=deg_j[:], in_=attn_sb[:, j, :], axis=AX.X)
                    rinv_j = s1t.tile([P, 1], f32, tag="rinv_j")
                    nc.vector.reciprocal(out=rinv_j[:], in_=deg_j[:])
                    nc.vector.tensor_scalar_mul(m_sb[j][:], attn_sb[:, j, :], rinv_j[:])

                # --- AllGather #1 (emb xT) + split reload ---
                nc.scalar.dma_start(out=cce_in[:, :], in_=xTe_loc[:])
                nc.gpsimd.collective_compute(
                    "AllGather",
                    mybir.AluOpType.bypass,
                    replica_groups=[list(range(NCORES))],
                    ins=[cce_in[:].opt()],
                    outs=[cce_out[:].opt()],
                )
                cce_re = cce_out.rearrange("(c p) f -> p c f", p=P)
                nc.gpsimd.dma_start(out=xTe_lo[:], in_=cce_re[:, 0:4, :])
                nc.gpsimd.dma_start(out=xTe_hi[:], in_=cce_re[:, 4:8, :])

                # --- RWPE chain: Q_{t+1} = M^T Q_t, diag from rows 0..255 ---
                cur, nxt = qA, qB
                for t in range(KPE):
                    for i in range(4):
                        pq = psq.tile([P, 256], f32, tag="pq")
                        for j in range(4):
                            nc.tensor.matmul(
                                out=pq[:],
                                lhsT=m_sb[j][:, i * P : (i + 1) * P],
                                rhs=cur[j][:],
                                start=(j == 0),
                                stop=(j == 3),
                            )
                        if i < 2:
                            # diag of this 128-row block: E mask = eye slice
                            # (tensor_tensor_reduce crashes on HW; use mul+red)
                            dummy = s1t.tile([P, 256], f32, tag="ttr_dummy")
                            nc.vector.tensor_mul(dummy[:], pq[:], eye2[i][:])
                            nc.vector.reduce_sum(
                                out=pe_pad[:, i, t : t + 1], in_=dummy[:], axis=AX.X
                            )
                        if t < KPE - 1:
                            if i % 2 == 0:
                                nc.scalar.copy(out=nxt[i][:], in_=pq[:])
                            else:
                                nc.vector.tensor_copy(out=nxt[i][:], in_=pq[:])
                    cur, nxt = nxt, cur

                # pe transpose -> xT_pe_loc [128, 256] bf16 (+AllGather#2)
                xTp_loc = s1.tile([P, 256], bf16, name="xTp_loc")
                for lj in range(2):
                    pt2 = pst.tile([P, P], f32, tag="pt")
                    nc.tensor.transpose(
                        out=pt2[:], in_=pe_pad[:, lj, :], identity=ident[:]
                    )
                    nc.vector.tensor_copy(
                        out=xTp_loc[:, lj * P : (lj + 1) * P], in_=pt2[:]
                    )
                nc.scalar.dma_start(out=ccp_in[:, :], in_=xTp_loc[:])
                nc.gpsimd.collective_compute(
                    "AllGather",
                    mybir.AluOpType.bypass,
                    replica_groups=[list(range(NCORES))],
                    ins=[ccp_in[:].opt()],
                    outs=[ccp_out[:].opt()],
                )
                nc.gpsimd.dma_start(
                    out=xTp_all[:],
                    in_=ccp_out.rearrange("(c p) r -> p c r", p=P),
                )

            # ---------------- Stage 2: logits = xT.T @ w ----------------
            def lhsT_for(m, k):
                rc, lj = divmod(m, 2)
                if k < KE:
                    xte = xTe_lo if rc < 4 else xTe_hi
                    o0 = k * 256 + lj * P
                    return xte[:, rc % 4, o0 : o0 + P]
                return xTp_all[:, rc, lj * P : (lj + 1) * P]

            with (
                tc.tile_pool(name="ob", bufs=4) as obp,
                tc.tile_pool(name="ps2", bufs=8, space="PSUM") as ps2,
            ):
                cp_i = 0

                def copyout(po, m, n0, ntn):
                    nonlocal cp_i
                    ob = obp.tile([P, 512], bf16, tag="ob")
                    if cp_i % 2 == 0:
                        nc.scalar.copy(out=ob[:, 0:ntn], in_=po[:, 0:ntn])
                    else:
                        nc.vector.tensor_copy(out=ob[:, 0:ntn], in_=po[:, 0:ntn])
                    cp_i += 1
                    nc.sync.dma_start(
                        out=logit_out[m * P : (m + 1) * P, n0 : n0 + ntn],
                        in_=ob[:, 0:ntn],
                    )

                for n, ntn in enumerate(ntiles):
                    n0 = n * 512
                    if n == 0:
                        # first v-tile: run k0..5 for 8 token tiles before any
                        # k6, so the PE isn't stalled on the (late) pe
                        # AllGather when the GEMM starts.
                        holds = []
                        for m in range(8):
                            po = ps2.tile([P, 512], f32, tag="po")
                            for k in range(KE):
                                nc.tensor.matmul(
                                    out=po[:, 0:ntn],
                                    lhsT=lhsT_for(m, k),
                                    rhs=w_k[k][:, n0 : n0 + ntn],
                                    start=(k == 0),
                                    stop=False,
                                )
                            holds.append(po)
                        for m in range(8):
                            po = holds[m]
                            nc.tensor.matmul(
                                out=po[:, 0:ntn],
                                lhsT=lhsT_for(m, KE),
                                rhs=w_k[KE][:, n0 : n0 + ntn],
                                start=False,
                                stop=True,
                            )
                            copyout(po, m, n0, ntn)
                        rest = range(8, MT)
                    else:
                        rest = range(MT)
                    for m in rest:
                        po = ps2.tile([P, 512], f32, tag="po")
                        for k in range(KCH):
                            nc.tensor.matmul(
                                out=po[:, 0:ntn],
                                lhsT=lhsT_for(m, k),
                                rhs=w_k[k][:, n0 : n0 + ntn],
                                start=(k == 0),
                                stop=(k == KCH - 1),
                            )
                        copyout(po, m, n0, ntn)

    nc.compile()
    return nc


def _host_prep(code_inputs, position_idx, attn_mask, emb_table, w_lin, b_lin, vc=VC):
    import ml_dtypes

    code = np.asarray(code_inputs).astype(np.int32)
    pos = np.asarray(position_idx).astype(np.int32)
    attn = np.asarray(attn_mask).astype(np.float32)
    emb_t = np.ascontiguousarray(np.asarray(emb_table, dtype=np.float32))
    w = np.asarray(w_lin, dtype=np.float32)
    bias = np.asarray(b_lin, dtype=np.float32)

    w_ext = np.zeros((DPAD, NCORES * vc), np.float32)
    ncols = min(NCORES * vc, V)
    w_ext[: D + KPE, :ncols] = w[:, :ncols]
    w_ext[D + KPE, :ncols] = bias[:ncols]
    w_ext = w_ext.astype(ml_dtypes.bfloat16)

    nodes = (pos == 0).astype(np.float32)
    token = (pos >= 2).astype(np.float32)

    in_maps = []
    for c in range(NCORES):
        b, h = divmod(c, 2)
        if h == 0:
            perm = np.arange(S)
        else:
            perm = np.r_[256:512, 0:256]
        a_p = attn[b][perm][:, perm]
        # [896, vc] -> [128 p, 7 k * vc] partition-major for contiguous DMA
        w_c = (
            w_ext[:, c * vc : (c + 1) * vc]
            .reshape(KCH, P, vc)
            .transpose(1, 0, 2)
            .reshape(P, KCH * vc)
        )
        in_maps.append(
            {
                "attn": np.ascontiguousarray(a_p),
                "attnT": np.ascontiguousarray(a_p[:256, :].T),
                "codes": np.ascontiguousarray(code[b][perm].reshape(4, P).T),
                "token_ps": np.ascontiguousarray(token[b][perm].reshape(4, P).T),
                "nodes_ps": np.ascontiguousarray(
                    nodes[b][perm][:256].reshape(2, P).T
                ),
                "emb_table": emb_t,
                "w_pad": np.ascontiguousarray(w_c),
            }
        )
    return in_maps


def run(inputs, trace=False, vc=VC, **run_kwargs):
    from concourse.bass_utils import run_bass_kernel_spmd

    key = ("nc", vc)
    nc = _CACHE.get(key)
    if nc is None:
        nc = _build_nc(vc=vc)
        _CACHE[key] = nc
    in_maps = _host_prep(**inputs, vc=vc)
    res = run_bass_kernel_spmd(
        nc, in_maps, core_ids=list(range(NCORES)), trace=trace, **run_kwargs
    )
    ncols = min(NCORES * vc, V)
    logits = np.concatenate(
        [np.asarray(r["logit"]).astype(np.float32) for r in res.results], axis=1
    )[:, :ncols]
    return logits.reshape(B, S, ncols), res


def kernel(**inputs):
    logits, _ = run(inputs, trace=False)
    return logits
